# revision 29
# baseline (speedup 1.0000x reference)
"""Per-sample 256-bin histogram -> broadcast [B,256,256], Trainium2 Bass kernel.

Input : x int32 [64, 786432], values in [0, 256)
Output: f32 [64, 256, 256] where out[b, i, j] = count(x[b, :] == i)

Sharding: pure data parallel, 8 rows per core across 8 NeuronCores.

Per-core algorithm (nibble decomposition + paired outer products):
  hist[16h + l] = sum_n onehot16(x_n >> 4)[h] * onehot16(x_n & 15)[l]
  The 32 one-hot channel planes per tile are the bandwidth wall, so their
  generation is split across three engines (all int16/bf16 ops run in the
  DVE 4x perf mode; every op form here passes the walrus birverifier,
  which rejects e.g. mixed bitwise+arith dual-op tensor_scalar):
  - ACT: narrows x int32 -> int16 (values < 256), computes 3 h-channels
    as exact hats Relu(1 - (h-a)^2) (Square + Relu, 2 ops each), and the
    epilogue broadcast multiplies.
  - DVE: h16 = x16 >> 4, l16 = x16 & 15, one bf16 cast of h16 for ACT,
    then ~20 single-op is_equal masks (327 ns each).
  - GPSIMD: 8-9 single-op is_equal masks (853 ns each), rotating one
    channel back to DVE every 8th tile to balance fractionally.
  - PE accumulates [32,32] PSUM outer products where the two diagonal
    [16,16] blocks are the valid per-half histograms (off-diagonal blocks
    are cross-half garbage that is simply never read). 256 elements per
    matmul instruction (~13.3 ns), the PE streaming cap.
  - Epilogue per row: 4 partition-scatter DMAs gather the two diagonal
    blocks into [128, 2, 2] columns, one DVE add folds the halves, then
    two ACT broadcast multiplies and two 128KB DMAs write out[r].
  Counts are integer-exact in f32 (max 786432 < 2^24).
"""

import sys

import numpy as np

sys.path.insert(0, "/opt/trn_rl_repo")

B = 64
N = 786432
NCORES = 8
ROWS_PER_CORE = B // NCORES
LEVELS = 256
P = 128

# Tile geometry: T columns per tile -> P*T elements per tile, in two halves.
T = 1024
T2 = T // 2
TILES = N // (P * T)
assert TILES * P * T == N

# Mask-channel assignment (32 channels = 16 h + 16 l):
ACT_H = (13, 14, 15)          # ACT hats
GPS_H = (9, 10, 11, 12)       # GPSIMD h-channels
GPS_L = (11, 12, 13, 14, 15)  # GPSIMD l-channels (l=11 moves to DVE 1-in-8)

_cache = {}


def _build_program(rows=None):
    import concourse.bacc as bacc
    from concourse import mybir
    from concourse import tile

    alu = mybir.AluOpType
    dt = mybir.dt
    af = mybir.ActivationFunctionType

    rows = ROWS_PER_CORE if rows is None else rows

    nc = bacc.Bacc(
        "TRN2",
        target_bir_lowering=False,
        debug=False,
        num_devices=NCORES,
    )
    x_dram = nc.dram_tensor("x", [rows, N], dt.int32, kind="ExternalInput")
    out_dram = nc.dram_tensor(
        "out", [rows, LEVELS, LEVELS], dt.float32, kind="ExternalOutput"
    )

    xv = x_dram.ap().rearrange("r (t p f) -> r t p f", p=P, f=T)
    ov = out_dram.ap()

    # Preamble const APs (barrier-ordered before all tile work): hat biases
    # for ACT Square, and the ones tile for the broadcast epilogue.
    for a in ACT_H:
        t_ = nc.alloc_sbuf_tensor(f"const-f32-m{a}", [128, 1], dt.float32)
        nc.gpsimd.memset(t_.ap(), float(-a))
        nc.const_aps.aps[(dt.float32, float(-a))] = t_.ap()
    ones_sb = nc.alloc_sbuf_tensor("ones_bcast", [P, LEVELS], dt.float32)
    nc.gpsimd.memset(ones_sb.ap(), 1.0)
    nc.all_engine_barrier()
    ones_ap = ones_sb.ap()

    with tile.TileContext(nc) as tc:
        with (
            tc.tile_pool(name="xin", bufs=4) as xpool,
            tc.tile_pool(name="hl", bufs=3) as hlpool,
            tc.tile_pool(name="mask", bufs=2) as mpool,
            tc.tile_pool(name="acc", bufs=2, space="PSUM") as ppool,
            tc.tile_pool(name="epi", bufs=3) as epool,
            tc.tile_pool(name="sq", bufs=3) as sqpool,
        ):

            def row_body(r):
                psum_hist = ppool.tile([32, 32], dt.float32, tag="psum_hist")
                for t in range(TILES):
                    xin = xv[r, t]
                    xt = xpool.tile([P, T], dt.int32, tag="xt")
                    # split across DMA queues for bandwidth
                    qs = T // 4
                    for q in range(4):
                        nc.sync.dma_start(
                            out=xt[:, q * qs : (q + 1) * qs],
                            in_=xin[..., q * qs : (q + 1) * qs],
                        )

                    # narrow to int16 on ACT (values < 256 fit exactly)
                    x16 = hlpool.tile([P, T], dt.int16, tag="x16")
                    nc.scalar.activation(out=x16[:], in_=xt[:], func=af.Copy)

                    # nibble extraction on DVE (single bitwise ops, 4x mode)
                    h16 = hlpool.tile([P, T], dt.int16, tag="h16")
                    nc.vector.tensor_scalar(
                        out=h16[:], in0=x16[:], scalar1=4, scalar2=None,
                        op0=alu.logical_shift_right,
                    )
                    l16 = hlpool.tile([P, T], dt.int16, tag="l16")
                    nc.vector.tensor_scalar(
                        out=l16[:], in0=x16[:], scalar1=15, scalar2=None,
                        op0=alu.bitwise_and,
                    )
                    h16v = h16[:].rearrange("p (g f) -> p g f", g=2)
                    l16v = l16[:].rearrange("p (g f) -> p g f", g=2)

                    hm = mpool.tile([P, 2, 16, T2], dt.bfloat16, tag="hm")
                    lm = mpool.tile([P, 2, 16, T2], dt.bfloat16, tag="lm")
                    gi = r * TILES + t
                    gps_l = GPS_L[1:] if gi % 8 in (2, 5, 7) else GPS_L

                    sq = sqpool.tile([P, 2, T2], dt.bfloat16, tag="sq")
                    for a in ACT_H:
                        # exact hat on ACT: Relu(1 - (h-a)^2), two ops
                        nc.scalar.activation(
                            out=sq[:], in_=h16v, func=af.Square,
                            bias=float(-a),
                        )
                        nc.scalar.activation(
                            out=hm[:, :, a, :], in_=sq[:], func=af.Relu,
                            scale=-1.0, bias=1.0,
                        )
                    for a in range(16):
                        if a in ACT_H:
                            continue
                        eng = nc.gpsimd if a in GPS_H else nc.vector
                        eng.tensor_scalar(
                            out=hm[:, :, a, :], in0=h16v,
                            scalar1=a, scalar2=None, op0=alu.is_equal,
                        )
                    for b in range(16):
                        eng = nc.gpsimd if b in gps_l else nc.vector
                        eng.tensor_scalar(
                            out=lm[:, :, b, :], in0=l16v,
                            scalar1=b, scalar2=None, op0=alu.is_equal,
                        )

                    for c in range(T2):
                        nc.tensor.matmul(
                            out=psum_hist[:],
                            lhsT=hm[:, :, :, c],
                            rhs=lm[:, :, :, c],
                            start=(t == 0 and c == 0),
                            stop=(t == TILES - 1 and c == T2 - 1),
                        )

                # --- epilogue for row r ---
                hist32 = epool.tile([32, 32], dt.float32, tag="hist32")
                nc.vector.tensor_copy(out=hist32[:], in_=psum_hist[:])
                # gather diagonal blocks: histcol2[i, half, g] for i = 16h+l
                histcol2 = epool.tile([P, 2, 2], dt.float32, tag="histcol2")
                nc.sync.dma_start(out=histcol2[:, 0, 0:1], in_=hist32[0:8, 0:16])
                nc.sync.dma_start(out=histcol2[:, 0, 1:2], in_=hist32[16:24, 16:32])
                nc.sync.dma_start(out=histcol2[:, 1, 0:1], in_=hist32[8:16, 0:16])
                nc.sync.dma_start(out=histcol2[:, 1, 1:2], in_=hist32[24:32, 16:32])
                histcol = epool.tile([P, 2], dt.float32, tag="histcol")
                nc.vector.tensor_tensor(
                    out=histcol[:], in0=histcol2[:, :, 0], in1=histcol2[:, :, 1],
                    op=alu.add,
                )

                for half in range(2):
                    bt = epool.tile([P, LEVELS], dt.float32, tag="bt")
                    nc.scalar.activation(
                        out=bt[:], in_=ones_ap, func=af.Copy,
                        scale=histcol[:, half : half + 1],
                    )
                    nc.sync.dma_start(
                        out=ov[r, half * P : (half + 1) * P, :], in_=bt[:]
                    )

            for r in range(rows):
                row_body(r)

    nc.compile()
    return nc


def _get_program(rows=None):
    key = ("nc", rows)
    if key not in _cache:
        _cache[key] = _build_program(rows)
    return _cache[key]


def kernel(x: np.ndarray) -> np.ndarray:
    from concourse.bass_utils import run_bass_kernel_spmd

    x = np.ascontiguousarray(np.asarray(x), dtype=np.int32)
    assert x.shape == (B, N), x.shape

    nc = _get_program()
    in_maps = [
        {"x": x[c * ROWS_PER_CORE : (c + 1) * ROWS_PER_CORE]} for c in range(NCORES)
    ]
    res = run_bass_kernel_spmd(nc, in_maps, core_ids=list(range(NCORES)))
    out = np.concatenate([res.results[c]["out"] for c in range(NCORES)], axis=0)
    return out.astype(np.float32)


# revision 33
# speedup vs baseline: 1.0769x; 1.0769x over previous
"""Per-sample 256-bin histogram -> broadcast [B,256,256], Trainium2 Bass kernel.

Input : x int32 [64, 786432], values in [0, 256)
Output: f32 [64, 256, 256] where out[b, i, j] = count(x[b, :] == i)

Sharding: pure data parallel, 8 rows per core across 8 NeuronCores.

Per-core algorithm (nibble decomposition + paired outer products):
  hist[16h + l] = sum_n onehot16(x_n >> 4)[h] * onehot16(x_n & 15)[l]
  Generating the per-tile channel planes is the bandwidth wall, so the
  work is split across three engines (int16/bf16 ops hit the DVE 4x perf
  mode; every op form here passes the walrus birverifier, which rejects
  e.g. mixed bitwise+arith dual-op tensor_scalar):
  - ACT: narrows x int32 -> int16, and computes 5 h-channels as
    RELU MOMENTS: plane M_r(a) = relu(h - a) for a in {11..15}, ONE
    activation op each. Since the second difference of relu is the
    delta function, hist(a) = M_r(a-1) - 2 M_r(a) + M_r(a+1) exactly
    (M_r(16) = 0), so planes {11..15} recover hist rows {12..15}.
    ACT also does the epilogue folds/broadcasts.
  - DVE: h16 = x16 >> 4, l16 = x16 & 15, then ~19-20 single-op
    is_equal one-hot masks (327 ns each): h in {0..11} minus GPS's.
  - GPSIMD: 8-9 single-op is_equal masks (853 ns each; one extra
    l-channel on every third tile for fractional balance).
  - PE accumulates [34,32] PSUM outer products; lhsT has 2 halves x 17
    h-planes (12 one-hot + 5 relu), rhs 2 x 16 l-planes. The two
    diagonal blocks are the valid per-half results (off-diagonal blocks
    are cross-half garbage, never read). Matmul cost scales only with
    the 32-wide moving dim, so the extra planes are free on PE.
  - Epilogue per row: partition-scatter DMAs gather the one-hot rows
    and the (prev, cur, next) relu-moment rows into per-(h,l) columns,
    ACT folds the halves and applies the second difference, then two
    ACT broadcast multiplies and two 128KB DMAs write out[r].
  Counts are integer-exact in f32 (all intermediates < 2^24).
"""

import sys

import numpy as np

sys.path.insert(0, "/opt/trn_rl_repo")

B = 64
N = 786432
NCORES = 8
ROWS_PER_CORE = B // NCORES
LEVELS = 256
P = 128

# Tile geometry: T columns per tile -> P*T elements per tile, in two halves.
T = 1024
T2 = T // 2
TILES = N // (P * T)
assert TILES * P * T == N

HP = 17  # h-planes per half: 12 one-hot (h 0..11) + 5 relu moments (a 11..15)
RELU_A = (11, 12, 13, 14, 15)   # plane 12+k holds relu(h - RELU_A[k])
GPS_H = (8, 9, 10, 11)          # GPSIMD one-hot h-channels
GPS_L = (12, 13, 14, 15)        # GPSIMD l-channels (+ l=11 on 1-in-3 tiles)

_cache = {}


def _build_program(rows=None):
    import concourse.bacc as bacc
    from concourse import mybir
    from concourse import tile

    alu = mybir.AluOpType
    dt = mybir.dt
    af = mybir.ActivationFunctionType

    rows = ROWS_PER_CORE if rows is None else rows

    nc = bacc.Bacc(
        "TRN2",
        target_bir_lowering=False,
        debug=False,
        num_devices=NCORES,
    )
    x_dram = nc.dram_tensor("x", [rows, N], dt.int32, kind="ExternalInput")
    out_dram = nc.dram_tensor(
        "out", [rows, LEVELS, LEVELS], dt.float32, kind="ExternalOutput"
    )

    xv = x_dram.ap().rearrange("r (t p f) -> r t p f", p=P, f=T)
    ov = out_dram.ap()

    # Preamble const APs (barrier-ordered before all tile work): relu biases
    # and the ones tile for the broadcast epilogue.
    for a in RELU_A:
        t_ = nc.alloc_sbuf_tensor(f"const-f32-m{a}", [128, 1], dt.float32)
        nc.gpsimd.memset(t_.ap(), float(-a))
        nc.const_aps.aps[(dt.float32, float(-a))] = t_.ap()
    ones_sb = nc.alloc_sbuf_tensor("ones_bcast", [P, LEVELS], dt.float32)
    nc.gpsimd.memset(ones_sb.ap(), 1.0)
    nc.all_engine_barrier()
    ones_ap = ones_sb.ap()

    with tile.TileContext(nc) as tc:
        with (
            tc.tile_pool(name="xin", bufs=4) as xpool,
            tc.tile_pool(name="hl", bufs=3) as hlpool,
            tc.tile_pool(name="mask", bufs=2) as mpool,
            tc.tile_pool(name="acc", bufs=2, space="PSUM") as ppool,
            tc.tile_pool(name="epi", bufs=3) as epool,
        ):

            def row_body(r):
                psum_hist = ppool.tile([2 * HP, 32], dt.float32, tag="psum_hist")
                for t in range(TILES):
                    xin = xv[r, t]
                    xt = xpool.tile([P, T], dt.int32, tag="xt")
                    # split across DMA queues for bandwidth
                    qs = T // 4
                    for q in range(4):
                        nc.sync.dma_start(
                            out=xt[:, q * qs : (q + 1) * qs],
                            in_=xin[..., q * qs : (q + 1) * qs],
                        )

                    # narrow to int16 on ACT (values < 256 fit exactly)
                    x16 = hlpool.tile([P, T], dt.int16, tag="x16")
                    nc.scalar.activation(out=x16[:], in_=xt[:], func=af.Copy)

                    # nibble extraction on DVE (single bitwise ops, 4x mode)
                    h16 = hlpool.tile([P, T], dt.int16, tag="h16")
                    nc.vector.tensor_scalar(
                        out=h16[:], in0=x16[:], scalar1=4, scalar2=None,
                        op0=alu.logical_shift_right,
                    )
                    l16 = hlpool.tile([P, T], dt.int16, tag="l16")
                    nc.vector.tensor_scalar(
                        out=l16[:], in0=x16[:], scalar1=15, scalar2=None,
                        op0=alu.bitwise_and,
                    )
                    h16v = h16[:].rearrange("p (g f) -> p g f", g=2)
                    l16v = l16[:].rearrange("p (g f) -> p g f", g=2)

                    hm = mpool.tile([P, 2, HP, T2], dt.bfloat16, tag="hm")
                    lm = mpool.tile([P, 2, 16, T2], dt.bfloat16, tag="lm")
                    gi = r * TILES + t
                    gps_l = GPS_L + (11,) if gi % 3 == 2 else GPS_L

                    # relu-moment planes on ACT: plane 12+k = relu(h - a_k)
                    for k, a in enumerate(RELU_A):
                        nc.scalar.activation(
                            out=hm[:, :, 12 + k, :], in_=h16v, func=af.Relu,
                            bias=float(-a),
                        )
                    # one-hot h planes 0..11
                    for a in range(12):
                        eng = nc.gpsimd if a in GPS_H else nc.vector
                        eng.tensor_scalar(
                            out=hm[:, :, a, :], in0=h16v,
                            scalar1=a, scalar2=None, op0=alu.is_equal,
                        )
                    for b in range(16):
                        eng = nc.gpsimd if b in gps_l else nc.vector
                        eng.tensor_scalar(
                            out=lm[:, :, b, :], in0=l16v,
                            scalar1=b, scalar2=None, op0=alu.is_equal,
                        )

                    for c in range(T2):
                        nc.tensor.matmul(
                            out=psum_hist[:],
                            lhsT=hm[:, :, :, c],
                            rhs=lm[:, :, :, c],
                            start=(t == 0 and c == 0),
                            stop=(t == TILES - 1 and c == T2 - 1),
                        )

                # --- epilogue for row r ---
                # hist34 rows (per g-block of 17): 0..11 one-hot h, 12..16
                # relu moments M_r(11..15). Valid cols: g0 0..15, g1 16..31.
                hist34 = epool.tile([2 * HP, 32], dt.float32, tag="hist34")
                nc.vector.tensor_copy(out=hist34[:], in_=psum_hist[:])

                # one-hot gathers: histcol2[p, half, g], p = 16*(h%8) + l
                histcol2 = epool.tile([P, 2, 2], dt.float32, tag="histcol2")
                # half 0: h 0..7  (rows 0..7 of each block)
                nc.sync.dma_start(out=histcol2[:, 0, 0:1], in_=hist34[0:8, 0:16])
                nc.sync.dma_start(out=histcol2[:, 0, 1:2], in_=hist34[HP : HP + 8, 16:32])
                # half 1, p 0..63: one-hot h 8..11 (rows 8..11)
                nc.sync.dma_start(out=histcol2[0:64, 1, 0:1], in_=hist34[8:12, 0:16])
                nc.sync.dma_start(out=histcol2[0:64, 1, 1:2], in_=hist34[HP + 8 : HP + 12, 16:32])

                # relu-moment gathers for h 12..15 -> half-1 partitions 64..127
                # cur  = M_r(h)   = plane h+1  (rows 13..16)
                # prev = M_r(h-1) = plane h    (rows 12..15)
                # next = M_r(h+1) = plane h+2  (rows 14..16; h=15 -> 0)
                gmom = epool.tile([64, 3, 2], dt.float32, tag="gmom")
                nc.vector.memset(gmom[:], 0.0)
                for g, (r0, c0) in enumerate(((0, 0), (HP, 16))):
                    nc.sync.dma_start(
                        out=gmom[0:64, 0, g : g + 1],
                        in_=hist34[r0 + 13 : r0 + 17, c0 : c0 + 16],
                    )
                    nc.sync.dma_start(
                        out=gmom[0:64, 1, g : g + 1],
                        in_=hist34[r0 + 12 : r0 + 16, c0 : c0 + 16],
                    )
                    nc.sync.dma_start(
                        out=gmom[0:48, 2, g : g + 1],
                        in_=hist34[r0 + 14 : r0 + 17, c0 : c0 + 16],
                    )

                histcol = epool.tile([P, 2], dt.float32, tag="histcol")
                # fold halves for one-hot partitions
                nc.vector.tensor_tensor(
                    out=histcol[:, 0:1], in0=histcol2[:, 0, 0:1],
                    in1=histcol2[:, 0, 1:2], op=alu.add,
                )
                nc.vector.tensor_tensor(
                    out=histcol[0:64, 1:2], in0=histcol2[0:64, 1, 0:1],
                    in1=histcol2[0:64, 1, 1:2], op=alu.add,
                )
                # relu part: fold g then second difference prev - 2 cur + next
                momf = epool.tile([64, 3], dt.float32, tag="momf")
                nc.vector.tensor_tensor(
                    out=momf[:], in0=gmom[:, :, 0], in1=gmom[:, :, 1], op=alu.add,
                )
                pn = epool.tile([64, 1], dt.float32, tag="pn")
                nc.vector.tensor_tensor(
                    out=pn[:], in0=momf[:, 1:2], in1=momf[:, 2:3], op=alu.add,
                )
                # histcol[64:128, 1] = pn - 2*cur   (ACT: -2*cur + bias(pn))
                nc.scalar.activation(
                    out=histcol[64:128, 1:2], in_=momf[:, 0:1], func=af.Identity,
                    scale=-2.0, bias=pn[:],
                )

                for half in range(2):
                    bt = epool.tile([P, LEVELS], dt.float32, tag="bt")
                    nc.scalar.activation(
                        out=bt[:], in_=ones_ap, func=af.Copy,
                        scale=histcol[:, half : half + 1],
                    )
                    nc.sync.dma_start(
                        out=ov[r, half * P : (half + 1) * P, :], in_=bt[:]
                    )

            for r in range(rows):
                row_body(r)

    nc.compile()
    return nc


def _get_program(rows=None):
    key = ("nc", rows)
    if key not in _cache:
        _cache[key] = _build_program(rows)
    return _cache[key]


def kernel(x: np.ndarray) -> np.ndarray:
    from concourse.bass_utils import run_bass_kernel_spmd

    x = np.ascontiguousarray(np.asarray(x), dtype=np.int32)
    assert x.shape == (B, N), x.shape

    nc = _get_program()
    in_maps = [
        {"x": x[c * ROWS_PER_CORE : (c + 1) * ROWS_PER_CORE]} for c in range(NCORES)
    ]
    res = run_bass_kernel_spmd(nc, in_maps, core_ids=list(range(NCORES)))
    out = np.concatenate([res.results[c]["out"] for c in range(NCORES)], axis=0)
    return out.astype(np.float32)


# revision 36
# speedup vs baseline: 1.0888x; 1.0111x over previous
"""Per-sample 256-bin histogram -> broadcast [B,256,256], Trainium2 Bass kernel.

Input : x int32 [64, 786432], values in [0, 256)
Output: f32 [64, 256, 256] where out[b, i, j] = count(x[b, :] == i)

Sharding: pure data parallel, 8 rows per core across 8 NeuronCores.

Per-core algorithm (nibble decomposition + paired outer products):
  hist[16h + l] = sum_n onehot16(x_n >> 4)[h] * onehot16(x_n & 15)[l]
  Generating the per-tile channel planes is the bandwidth wall, so the
  work is split across three engines (int16/bf16 ops hit the DVE 4x perf
  mode; every op form here passes the walrus birverifier, which rejects
  e.g. mixed bitwise+arith dual-op tensor_scalar):
  - ACT: narrows x int32 -> int16, and computes 5 h-channels as
    RELU MOMENTS: plane M_r(a) = relu(h - a) for a in {11..15}, ONE
    activation op each. Since the second difference of relu is the
    delta function, hist(a) = M_r(a-1) - 2 M_r(a) + M_r(a+1) exactly
    (M_r(16) = 0), so planes {11..15} recover hist rows {12..15}.
    ACT also does the epilogue folds/broadcasts.
  - DVE: h16 = x16 >> 4, l16 = x16 & 15, then ~19-20 single-op
    is_equal one-hot masks (327 ns each): h in {0..11} minus GPS's.
  - GPSIMD: 8-9 single-op is_equal masks (853 ns each; one extra
    l-channel on every third tile for fractional balance).
  - PE accumulates [34,32] PSUM outer products; lhsT has 2 halves x 17
    h-planes (12 one-hot + 5 relu), rhs 2 x 16 l-planes. The two
    diagonal blocks are the valid per-half results (off-diagonal blocks
    are cross-half garbage, never read). Matmul cost scales only with
    the 32-wide moving dim, so the extra planes are free on PE.
  - Epilogue per row: partition-scatter DMAs gather the one-hot rows
    and the (prev, cur, next) relu-moment rows into per-(h,l) columns,
    ACT folds the halves and applies the second difference, then two
    ACT broadcast multiplies and two 128KB DMAs write out[r].
  Counts are integer-exact in f32 (all intermediates < 2^24).
"""

import sys

import numpy as np

sys.path.insert(0, "/opt/trn_rl_repo")

B = 64
N = 786432
NCORES = 8
ROWS_PER_CORE = B // NCORES
LEVELS = 256
P = 128

# Tile geometry: T columns per tile -> P*T elements per tile, in two halves.
T = 1024
T2 = T // 2
TILES = N // (P * T)
assert TILES * P * T == N

HP = 17  # h-planes per half: 12 one-hot (h 0..11) + 5 relu moments (a 11..15)
RELU_A = (11, 12, 13, 14, 15)   # plane 12+k holds relu(h - RELU_A[k])
GPS_H = (8, 9, 10, 11)          # GPSIMD one-hot h-channels
GPS_L = (12, 13, 14, 15)        # GPSIMD l-channels (+ l=11 on 1-in-3 tiles)

_cache = {}


def _build_program(rows=None):
    import concourse.bacc as bacc
    from concourse import mybir
    from concourse import tile

    alu = mybir.AluOpType
    dt = mybir.dt
    af = mybir.ActivationFunctionType

    rows = ROWS_PER_CORE if rows is None else rows

    nc = bacc.Bacc(
        "TRN2",
        target_bir_lowering=False,
        debug=False,
        num_devices=NCORES,
    )
    x_dram = nc.dram_tensor("x", [rows, N], dt.int32, kind="ExternalInput")
    out_dram = nc.dram_tensor(
        "out", [rows, LEVELS, LEVELS], dt.float32, kind="ExternalOutput"
    )

    xv = x_dram.ap().rearrange("r (t p f) -> r t p f", p=P, f=T)
    ov = out_dram.ap()

    # Preamble const APs (barrier-ordered before all tile work): relu biases
    # and the ones tile for the broadcast epilogue.
    for a in RELU_A:
        t_ = nc.alloc_sbuf_tensor(f"const-f32-m{a}", [128, 1], dt.float32)
        nc.gpsimd.memset(t_.ap(), float(-a))
        nc.const_aps.aps[(dt.float32, float(-a))] = t_.ap()
    ones_sb = nc.alloc_sbuf_tensor("ones_bcast", [P, LEVELS], dt.float32)
    nc.gpsimd.memset(ones_sb.ap(), 1.0)
    nc.all_engine_barrier()
    ones_ap = ones_sb.ap()

    with tile.TileContext(nc) as tc:
        with (
            tc.tile_pool(name="xin", bufs=4) as xpool,
            tc.tile_pool(name="hl", bufs=3) as hlpool,
            tc.tile_pool(name="mask", bufs=2) as mpool,
            tc.tile_pool(name="acc", bufs=2, space="PSUM") as ppool,
            tc.tile_pool(name="epi", bufs=3) as epool,
        ):

            def tile_chunks(r, t):
                # shorter pipeline fill/drain: split the very first and
                # very last tiles of the schedule into smaller chunks
                if r == 0 and t == 0:
                    return (256, 256, 512)
                if r == rows - 1 and t == TILES - 1:
                    return (512, 256, 256)
                return (T,)

            def row_body(r):
                psum_hist = ppool.tile([2 * HP, 32], dt.float32, tag="psum_hist")
                for t in range(TILES):
                    xin = xv[r, t]
                    chunks = tile_chunks(r, t)
                    off = 0
                    for ci, W in enumerate(chunks):
                        W2 = W // 2
                        first = t == 0 and ci == 0
                        last = t == TILES - 1 and ci == len(chunks) - 1
                        xt = xpool.tile([P, T], dt.int32, tag="xt")
                        nq = max(1, W // 256)
                        qs = W // nq
                        for q in range(nq):
                            nc.sync.dma_start(
                                out=xt[:, q * qs : (q + 1) * qs],
                                in_=xin[..., off + q * qs : off + (q + 1) * qs],
                            )

                        # narrow to int16 on ACT (values < 256 fit exactly)
                        x16 = hlpool.tile([P, T], dt.int16, tag="x16")
                        nc.scalar.activation(
                            out=x16[:, 0:W], in_=xt[:, 0:W], func=af.Copy
                        )

                        # nibble extraction on DVE (single bitwise ops, 4x)
                        h16 = hlpool.tile([P, T], dt.int16, tag="h16")
                        nc.vector.tensor_scalar(
                            out=h16[:, 0:W], in0=x16[:, 0:W], scalar1=4,
                            scalar2=None, op0=alu.logical_shift_right,
                        )
                        l16 = hlpool.tile([P, T], dt.int16, tag="l16")
                        nc.vector.tensor_scalar(
                            out=l16[:, 0:W], in0=x16[:, 0:W], scalar1=15,
                            scalar2=None, op0=alu.bitwise_and,
                        )
                        h16v = h16[:, 0:W].rearrange("p (g f) -> p g f", g=2)
                        l16v = l16[:, 0:W].rearrange("p (g f) -> p g f", g=2)

                        hm = mpool.tile([P, 2, HP, T2], dt.bfloat16, tag="hm")
                        lm = mpool.tile([P, 2, 16, T2], dt.bfloat16, tag="lm")
                        gi = r * TILES + t
                        gps_l = GPS_L + (11,) if gi % 3 == 2 else GPS_L

                        # relu-moment planes on ACT: plane 12+k = relu(h-a_k)
                        for k, a in enumerate(RELU_A):
                            nc.scalar.activation(
                                out=hm[:, :, 12 + k, 0:W2], in_=h16v,
                                func=af.Relu, bias=float(-a),
                            )
                        # one-hot h planes 0..11
                        for a in range(12):
                            eng = nc.gpsimd if a in GPS_H else nc.vector
                            eng.tensor_scalar(
                                out=hm[:, :, a, 0:W2], in0=h16v,
                                scalar1=a, scalar2=None, op0=alu.is_equal,
                            )
                        for b in range(16):
                            eng = nc.gpsimd if b in gps_l else nc.vector
                            eng.tensor_scalar(
                                out=lm[:, :, b, 0:W2], in0=l16v,
                                scalar1=b, scalar2=None, op0=alu.is_equal,
                            )

                        for c in range(W2):
                            nc.tensor.matmul(
                                out=psum_hist[:],
                                lhsT=hm[:, :, :, c],
                                rhs=lm[:, :, :, c],
                                start=(first and c == 0),
                                stop=(last and c == W2 - 1),
                            )
                        off += W

                # --- epilogue for row r ---
                # hist34 rows (per g-block of 17): 0..11 one-hot h, 12..16
                # relu moments M_r(11..15). Valid cols: g0 0..15, g1 16..31.
                hist34 = epool.tile([2 * HP, 32], dt.float32, tag="hist34")
                nc.scalar.activation(out=hist34[:], in_=psum_hist[:], func=af.Copy)

                # On the last row every other engine is idle (pipeline
                # drain), so spread descriptor generation across queues.
                last_row = r == rows - 1
                dq1 = nc.gpsimd if last_row else nc.sync
                dq2 = nc.scalar if last_row else nc.sync
                # one-hot gathers: histcol2[p, half, g], p = 16*(h%8) + l
                histcol2 = epool.tile([P, 2, 2], dt.float32, tag="histcol2")
                # half 0: h 0..7  (rows 0..7 of each block)
                nc.sync.dma_start(out=histcol2[:, 0, 0:1], in_=hist34[0:8, 0:16])
                dq1.dma_start(out=histcol2[:, 0, 1:2], in_=hist34[HP : HP + 8, 16:32])
                # half 1, p 0..63: one-hot h 8..11 (rows 8..11)
                dq2.dma_start(out=histcol2[0:64, 1, 0:1], in_=hist34[8:12, 0:16])
                nc.sync.dma_start(out=histcol2[0:64, 1, 1:2], in_=hist34[HP + 8 : HP + 12, 16:32])

                # relu-moment gathers for h 12..15 -> half-1 partitions 64..127
                # cur  = M_r(h)   = plane h+1  (rows 13..16)
                # prev = M_r(h-1) = plane h    (rows 12..15)
                # next = M_r(h+1) = plane h+2  (rows 14..16; h=15 -> 0)
                gmom = epool.tile([64, 3, 2], dt.float32, tag="gmom")
                nc.gpsimd.memset(gmom[:], 0.0)
                for g, (r0, c0) in enumerate(((0, 0), (HP, 16))):
                    dq1.dma_start(
                        out=gmom[0:64, 0, g : g + 1],
                        in_=hist34[r0 + 13 : r0 + 17, c0 : c0 + 16],
                    )
                    dq2.dma_start(
                        out=gmom[0:64, 1, g : g + 1],
                        in_=hist34[r0 + 12 : r0 + 16, c0 : c0 + 16],
                    )
                    nc.sync.dma_start(
                        out=gmom[0:48, 2, g : g + 1],
                        in_=hist34[r0 + 14 : r0 + 17, c0 : c0 + 16],
                    )

                histcol = epool.tile([P, 2], dt.float32, tag="histcol")
                # fold halves for one-hot partitions
                nc.scalar.activation(
                    out=histcol[:, 0:1], in_=histcol2[:, 0, 0:1],
                    func=af.Identity, bias=histcol2[:, 0, 1:2],
                )
                nc.scalar.activation(
                    out=histcol[0:64, 1:2], in_=histcol2[0:64, 1, 0:1],
                    func=af.Identity, bias=histcol2[0:64, 1, 1:2],
                )
                # relu part: fold g then second difference prev - 2 cur + next
                momf = epool.tile([64, 3], dt.float32, tag="momf")
                nc.vector.tensor_tensor(
                    out=momf[:], in0=gmom[:, :, 0], in1=gmom[:, :, 1], op=alu.add,
                )
                pn = epool.tile([64, 1], dt.float32, tag="pn")
                nc.scalar.activation(
                    out=pn[:], in_=momf[:, 1:2], func=af.Identity,
                    bias=momf[:, 2:3],
                )
                # histcol[64:128, 1] = pn - 2*cur   (ACT: -2*cur + bias(pn))
                nc.scalar.activation(
                    out=histcol[64:128, 1:2], in_=momf[:, 0:1], func=af.Identity,
                    scale=-2.0, bias=pn[:],
                )

                for half in range(2):
                    bt = epool.tile([P, LEVELS], dt.float32, tag="bt")
                    nc.scalar.activation(
                        out=bt[:], in_=ones_ap, func=af.Copy,
                        scale=histcol[:, half : half + 1],
                    )
                    (dq1 if half else nc.sync).dma_start(
                        out=ov[r, half * P : (half + 1) * P, :], in_=bt[:]
                    )

            for r in range(rows):
                row_body(r)

    nc.compile()
    return nc


def _get_program(rows=None):
    key = ("nc", rows)
    if key not in _cache:
        _cache[key] = _build_program(rows)
    return _cache[key]


def kernel(x: np.ndarray) -> np.ndarray:
    from concourse.bass_utils import run_bass_kernel_spmd

    x = np.ascontiguousarray(np.asarray(x), dtype=np.int32)
    assert x.shape == (B, N), x.shape

    nc = _get_program()
    in_maps = [
        {"x": x[c * ROWS_PER_CORE : (c + 1) * ROWS_PER_CORE]} for c in range(NCORES)
    ]
    res = run_bass_kernel_spmd(nc, in_maps, core_ids=list(range(NCORES)))
    out = np.concatenate([res.results[c]["out"] for c in range(NCORES)], axis=0)
    return out.astype(np.float32)
